# revision 39
# baseline (speedup 1.0000x reference)
"""MoE kernel for Trainium2 (8 NeuronCores, expert-parallel, fp8 DoubleRow).

Strategy
--------
N=8192 tokens, D=1024, E=8 experts, DFF=4096, top_k=2. The reference
computes every expert densely and masks; only each token's top-2 experts
contribute, so we dispatch each token to its 2 experts and run the
expert MLPs on just the routed tokens: 4x fewer FLOPs than dense.

Load balance: expert loads are uneven (1932..2182 here), so instead of
one expert per core (which pads every core to the straggler's 2304
tokens), each core runs K weight slots with compile-time sizes
sum(sizes)=S. The host solves a small covering problem (DP) for the
minimal S such that all experts' token lists pack into 8 bins per slot
class (each bin single-expert); K=3 lands at S=2064 vs the perfect
2048 vs the naive 2304 (-10% PE time).

fp8 DoubleRow matmuls: the PE contracts 2 k-chunks (256 rows) per
DoubleRow instruction at 0.5 cycles per moving row, so a hi+lo fp8
decomposition a@b ~= ah@bh + ah@bl + al@bh (3 e4m3 product terms, the
ll term is ~7e-4 and dropped) runs the same math in 0.75x the bf16
cycles with ~2.7e-3 end-to-end error (bf16 baseline: 3.4e-3). All
hi/lo pairs share one power-of-2 scale per tensor so every term can
accumulate into the same PSUM group: x*32, w1*2048, h*1, w2*2048.
The 2^-16 PSUM scale of matmul 1 is folded into the Silu activation's
input scale; the 2^-11 of matmul 2 is folded into the host-computed
gate weights. h is split on-chip: Silu->bf16 (scalar), Copy->fp8 hh
(scalar), hl = h - hh (DVE, fp8 out).

Both matmuls keep tokens on the PE free dim (phase A: h^T[f,t], phase
B: y^T[d,t]), so block sizes are exact token counts - no 128-row
padding anywhere. The gate weight is applied with a DVE elementwise
multiply against a partition-broadcast copy of the combine weights.

Weights are loaded into SBUF once per slot (graded chunk sizes in
consumption order: small first so the first matmuls start ~5us in,
large after for DMA efficiency; each next slot's load overlaps the
previous slot's trailing phase B). Phase A of the first two blocks is
fused per-mf so the w1 stream keeps up.

Host (unshard): y[token] = yT[core1][:, col1] + yT[core2][:, col2].
"""

import numpy as np

import concourse.bass as bass
import concourse.bacc as bacc
import concourse.tile as tile
from concourse import mybir
from concourse.bass_utils import run_bass_kernel_spmd

N, D, E, DFF = 8192, 1024, 8, 4096
P = 128
KD = D // P  # 8 k-chunks, first matmul
KF = DFF // P  # 32 k-chunks, second matmul
MD = D // P  # 8 output-row tiles, second matmul

# fp8 hi/lo scales (powers of 2; host asserts amax stays under 240)
SX = 32.0  # x scale
SW = 2048.0  # w1 and w2 scale
LOG2_SXW = 16  # log2(SX*SW): psum scale of matmul 1
LOG2_SW = 11  # log2(SW): psum scale of matmul 2

# Per-phase count of chunk-pairs computed with a single pure-fp8 term
# instead of the 3-term hi/lo scheme (speed/accuracy knob; each pure
# pair saves 2 DoubleRow instructions but adds ~5.4e-2*sqrt(frac) err).
PURE_A = 0  # of the 4 k-chunk pairs in matmul 1
PURE_B = 2  # of the 16 k-chunk pairs in matmul 2

TRACE = False
LAST_RESULT = None
LAST_NC = None
REPS = 1  # >1: repeat whole computation in-program (for slope timing)


def _chunks_even(total, maxb=512):
    """Split into near-equal blocks <= maxb, multiples of 16 (except possibly
    the last), avoiding tiny tail blocks that expose handoff latency."""
    nb = -(-total // maxb)
    out, rem = [], total
    for i in range(nb):
        b = min(rem, int(np.ceil(rem / (nb - i) / 16) * 16), maxb)
        out.append(b)
        rem -= b
    assert rem == 0 and sum(out) == total
    return out


def _feasible(counts, sizes, n_bins=E, parents=None):
    """DP: can counts be covered with n_bins bins of each size class?
    State: per-class bins used. If parents given, fill for backtracking."""
    K = len(sizes)
    reach = {tuple([0] * K)}
    for e, c in enumerate(counts):
        nxt = set()
        pe = {} if parents is not None else None
        for st in reach:

            def rec(k, st_k, rem):
                if rem <= 0:
                    key = tuple(st_k)
                    if key not in nxt:
                        nxt.add(key)
                        if pe is not None:
                            pe[key] = (st, tuple(np.subtract(st_k, st)))
                    return
                if k == K:
                    return
                for nk in range(n_bins - st_k[k] + 1):
                    st2 = list(st_k)
                    st2[k] += nk
                    rec(k + 1, st2, rem - nk * sizes[k])
                    if nk * sizes[k] >= rem:
                        break

            rec(0, list(st), c)
        if parents is not None:
            parents.append(pe)
        reach = nxt
        if not reach:
            return None
    return next(iter(reach))


def _optimize_slots(counts, n_bins=E):
    """Find slot sizes (K=2, or 3 if strictly better) minimizing
    S = sum(sizes). Returns (sizes, assign) with assign[e][k] = #bins of
    class k used by expert e."""
    counts = np.asarray(counts, dtype=int)
    lo = int(np.ceil(counts.sum() / n_bins / 16) * 16)
    hi = int(np.ceil(counts.max() / 16) * 16) + 16

    def slack_ok(S):
        # zero-slack S needs an exact cover by multiples of 16 => every
        # count must be divisible by 16 (cheap prune of the full scan)
        slack = n_bins * S - int(counts.sum())
        return slack > 0 or all(c % 16 == 0 for c in counts)

    best = None
    S2 = None
    for S in range(lo, 2 * hi, 16):
        if not slack_ok(S):
            continue
        for S_A in range(256, S // 2 + 1, 16):
            if _feasible(counts, (S_A, S - S_A)) is not None:
                best = (S_A, S - S_A)
                break
        if best:
            S2 = S
            break
    assert best is not None, "no 2-slot split found"

    found3 = None
    for S in range(lo, S2, 16):
        if not slack_ok(S):
            continue
        for S_A in range(256, S // 3 + 1, 16):
            for S_B in range(S_A, (S - S_A) // 2 + 1, 16):
                S_C = S - S_A - S_B
                if _feasible(counts, (S_A, S_B, S_C)) is not None:
                    found3 = (S_A, S_B, S_C)
                    break
            if found3:
                break
        if found3:
            break
    sizes = found3 if found3 is not None else best

    # Order slots to maximize the weight-reload windows: the reload of slot
    # k+1 overlaps slot k's LAST block's phase B, so prefer large last
    # blocks on the slots that precede a reload (and a large first slot for
    # the fused start).
    import itertools

    def min_window(order):
        wins = [_chunks_even(order[k])[-1] for k in range(len(order) - 1)]
        return min(wins) if wins else 1 << 30

    sizes = max(
        itertools.permutations(sizes), key=lambda o: (min_window(o), o[0])
    )

    parents = []
    assert _feasible(counts, sizes, n_bins, parents) is not None
    assign = [None] * len(counts)
    cur = next(iter(parents[-1]))
    for e in range(len(counts) - 1, -1, -1):
        prev, used = parents[e][cur]
        assign[e] = list(used)
        cur = prev
    return list(sizes), assign


def build_nc(sizes, reps=1):
    """Per-core program: yT[d, t] = wgt[t] * (silu(x @ w1) @ w2)[t, d]
    over len(sizes) weight slots, fp8 DoubleRow hi/lo matmuls."""
    f8 = mybir.dt.float8e4
    bf16 = mybir.dt.bfloat16
    f32 = mybir.dt.float32
    ACT = mybir.ActivationFunctionType
    DR = mybir.MatmulPerfMode.DoubleRow

    K = len(sizes)
    S = sum(sizes)
    nc = bacc.Bacc()
    # All fp8 hi/lo pairs are interleaved byte-wise in the innermost DRAM
    # dim ([..., 2]: 0=hi, 1=lo) so DMA inner contiguous runs stay >=512B
    # (the cost model charges 2x below 512B); matmul APs then read the
    # hi or lo plane with an innermost stride of 2.
    xq = nc.dram_tensor("xq", [D, 2 * S], f8, kind="ExternalInput")
    w1s = [
        nc.dram_tensor(f"w1_{k}", [D, 2 * DFF], f8, kind="ExternalInput")
        for k in range(K)
    ]
    # w2 split into d-halves (separate tensors so the two halves' SBUF
    # tiles free at different times, staggering the next slot's reload)
    DH = D // 2
    w2s = [
        [
            nc.dram_tensor(f"w2{h}_{k}", [DFF, 2 * DH], f8, kind="ExternalInput")
            for h in ("a", "b")
        ]
        for k in range(K)
    ]
    wgtb = nc.dram_tensor("wgtb", [P, S], f32, kind="ExternalInput")
    y = nc.dram_tensor("y", [D, S], bf16, kind="ExternalOutput")

    xq_r = xq.rearrange("(k p) (s two) -> p k s two", p=P, two=2)
    w1_rs = [
        w.rearrange("(k p) (f two) -> p k f two", p=P, two=2) for w in w1s
    ]
    w2_rs = [
        [w.rearrange("(kf p) (d two) -> p kf d two", p=P, two=2) for w in pair]
        for pair in w2s
    ]
    y_r = y.rearrange("(m p) s -> m p s", p=P)  # [8, 128, S]

    # compile-time block schedule: (slot, tok0, B). Slot 0 starts with a
    # small block (its phase A is fused with block 1's, so the PE can start
    # after one small xg DMA + the first w1 chunk). The very last block is
    # small so the end-of-kernel drain waits on a short mult+DMA.
    sched = []
    off = 0
    for s, S_s in enumerate(sizes):
        t0 = off
        if s == 0 and S_s > 512:
            # the fused first pair totals ~320 tokens: the PE consumption
            # rate of the fused phase A then matches the w1 DMA stream
            # rate (2KB/partition per mf chunk), so the start is
            # DMA-paced with no mid-stream stalls
            bs = [128, 192] + _chunks_even(S_s - 320)
        else:
            bs = _chunks_even(S_s)
        for b in bs:
            sched.append((s, t0, b))
            t0 += b
        off += S_s

    # per-pair term list: hi@hi always; full pairs add lo-weight and
    # lo-moving terms ((j_weight, j_moving) indices into the hi/lo dims)
    TERMS_FULL = ((0, 0), (1, 0), (0, 1))
    TERMS_PURE = ((0, 0),)

    def terms_a(cp):
        return TERMS_PURE if cp < PURE_A else TERMS_FULL

    def terms_b(kp):
        return TERMS_PURE if kp < PURE_B else TERMS_FULL

    with tile.TileContext(nc) as tc:
        with (
            tc.tile_pool(name="singles", bufs=1) as singles,
            tc.tile_pool(name="wres", bufs=1) as w_pool,
            tc.tile_pool(name="xg", bufs=3) as xg_pool,
            tc.tile_pool(name="ht", bufs=1) as h_pool,
            tc.tile_pool(name="ht0", bufs=1) as h0_pool,
            tc.tile_pool(name="hbf", bufs=3) as hbf_pool,
            tc.tile_pool(name="yout", bufs=3) as y_pool,
            tc.tile_pool(name="hps", bufs=5, space="PSUM") as hpsum,
            tc.tile_pool(name="yps", bufs=3, space="PSUM") as ypsum,
        ):
            xg_tiles = {}  # i -> (tile, col0)

            def load_xg(i, eng=None):
                _, tok0, B = sched[i]
                t = xg_pool.tile([P, KD, B, 2], f8, tag="xg", name="xg")
                (eng or nc.scalar).dma_start(
                    out=t, in_=xq_r[:, :, tok0 : tok0 + B, :]
                )
                xg_tiles[i] = (t, 0)

            def load_xg_pair(i, j, eng):
                """One DMA for two adjacent blocks (shared tile): fewer
                descriptors and a longer (full-rate) inner run."""
                _, t0i, Bi = sched[i]
                _, t0j, Bj = sched[j]
                assert t0j == t0i + Bi
                t = xg_pool.tile([P, KD, Bi + Bj, 2], f8, tag="xg", name="xg")
                eng.dma_start(out=t, in_=xq_r[:, :, t0i : t0i + Bi + Bj, :])
                xg_tiles[i] = (t, 0)
                xg_tiles[j] = (t, Bi)

            def load_weights(slot, xg_loader=None, wgt_late=False):
                """w1 halves first (phase A streams them; graded sizes:
                small first so the PE starts early, >=512B runs after for
                DMA efficiency), then the w2 d-halves (each needed in
                full only by its first phase-B md group); wgtb early in
                the w2 stream. The a/b halves are separate SBUF tiles so
                the next slot's reload of each half starts as soon as the
                previous slot's readers of that half are done (mf<16 /
                md<4 finish well before the slot ends). xg_loader: called
                right after w1's first chunk to slot in the first xg DMA
                (it gates the first matmuls; w1's chunk 0 gates only the
                Ldweights before them)."""
                HF = KF // 2
                w1_ts = []
                for h in range(2):
                    t = w_pool.tile(
                        [P, KD, HF * P, 2], f8, tag=f"w1{h}", name="w1"
                    )
                    # g=2 is the smallest full-rate chunk (512B runs);
                    # few big chunks keep descriptor-generation overhead
                    # (~0.6us per DMA) off the startup critical path
                    grades = [2, 2, 4, 8] if h == 0 else [8, 8]
                    mf = 0
                    for gi, g in enumerate(grades):
                        nc.sync.dma_start(
                            out=t[:, :, mf * P : (mf + g) * P, :],
                            in_=w1_rs[slot][
                                :,
                                :,
                                (h * HF + mf) * P : (h * HF + mf + g) * P,
                                :,
                            ],
                        )
                        mf += g
                        if h == 0 and gi == 0 and xg_loader is not None:
                            xg_loader()
                    assert mf == HF
                    w1_ts.append(t)
                w2_ts = []
                for h in range(2):
                    t = w_pool.tile(
                        [P, KF, DH, 2], f8, tag=f"w2{h}", name="w2"
                    )
                    for c in range(2):
                        nc.sync.dma_start(
                            out=t[:, c * 16 : (c + 1) * 16, :, :],
                            in_=w2_rs[slot][h][:, c * 16 : (c + 1) * 16, :, :],
                        )
                    if h == 0 and wgt_late:
                        nc.sync.dma_start(out=wgt_t, in_=wgtb[:, :])
                    w2_ts.append(t)
                return w1_ts, w2_ts

            def phase_a_multi(iis, w1_ts, pools, mf_hook=None):
                """Fused phase A over several blocks: per-mf across all
                blocks, so each w1 chunk is consumed at the combined rate
                (lets the first blocks start before w1 fully lands)."""
                xs = [xg_tiles.pop(i) for i in iis]
                hts = []
                for i, pool in zip(iis, pools):
                    _, _, B = sched[i]
                    hts.append(
                        pool.tile(
                            [P, 2, KF, B],
                            f8,
                            tag="ht0" if pool is h0_pool else "ht",
                            name="ht",
                        )
                    )

                def emit(bi, mf):
                    i = iis[bi]
                    _, _, B = sched[i]
                    xt, c0 = xs[bi]
                    w1_t = w1_ts[mf // (KF // 2)]
                    mfl = mf % (KF // 2)
                    ph = hpsum.tile([P, B], f32, tag="hps", name="hps")
                    n_mm = sum(len(terms_a(cp)) for cp in range(KD // 2))
                    mm = 0
                    for cp in range(KD // 2):
                        for jw, jx in terms_a(cp):
                            nc.tensor.matmul(
                                ph[:, :],
                                lhsT=w1_t[
                                    :,
                                    2 * cp : 2 * cp + 2,
                                    mfl * P : (mfl + 1) * P,
                                    jw,
                                ],
                                rhs=xt[
                                    :, 2 * cp : 2 * cp + 2, c0 : c0 + B, jx
                                ],
                                start=(mm == 0),
                                stop=(mm == n_mm - 1),
                                perf_mode=DR,
                            )
                            mm += 1
                    hbf = hbf_pool.tile([P, B], bf16, tag="hbf", name="hbf")
                    nc.scalar.activation(
                        hbf[:, :], ph[:, :], ACT.Silu, scale=2.0**-LOG2_SXW
                    )
                    nc.scalar.activation(
                        hts[bi][:, 0, mf, :], hbf[:, :], ACT.Copy
                    )
                    nc.vector.tensor_sub(
                        hts[bi][:, 1, mf, :], hbf[:, :], hts[bi][:, 0, mf, :]
                    )

                for mf in range(KF):
                    if mf_hook is not None:
                        mf_hook(mf)
                    for bi in range(len(iis)):
                        emit(bi, mf)
                return hts

            def phase_a(i, w1_ts, pool):
                return phase_a_multi([i], w1_ts, [pool])[0]

            def phase_b(i, w2_ts, hT):
                _, tok0, B = sched[i]
                for md in range(MD):
                    w2_t = w2_ts[md // (MD // 2)]
                    mdl = md % (MD // 2)
                    yp = ypsum.tile([P, B], f32, tag="yps", name="yps")
                    n_mm = sum(len(terms_b(kp)) for kp in range(KF // 2))
                    mm = 0
                    for kp in range(KF // 2):
                        for jw, jh in terms_b(kp):
                            nc.tensor.matmul(
                                yp[:, :],
                                lhsT=w2_t[
                                    :,
                                    2 * kp : 2 * kp + 2,
                                    mdl * P : (mdl + 1) * P,
                                    jw,
                                ],
                                rhs=hT[:, jh, 2 * kp : 2 * kp + 2, :],
                                start=(mm == 0),
                                stop=(mm == n_mm - 1),
                                perf_mode=DR,
                            )
                            mm += 1
                    y_sb = y_pool.tile([P, B], bf16, tag="yout", name="yout")
                    nc.vector.tensor_mul(
                        y_sb[:, :], yp[:, :], wgt_t[:, tok0 : tok0 + B]
                    )
                    nc.scalar.dma_start(
                        out=y_r[md, :, tok0 : tok0 + B], in_=y_sb[:, :]
                    )

            nblk = len(sched)
            wgt_t = singles.tile([P, S], f32)

            cur_slot = -1
            for rep in range(reps):
                i = 0
                while i < nblk:
                    slot, tok0, B = sched[i]
                    first = cur_slot == -1
                    if slot != cur_slot:
                        if first:
                            def ldr():
                                with tc.high_priority():
                                    if nblk > 1 and sched[1][0] == 0:
                                        load_xg_pair(0, 1, nc.sync)
                                    else:
                                        load_xg(0, eng=nc.sync)
                        else:
                            ldr = None
                        w1_ts, w2_ts = load_weights(
                            slot, xg_loader=ldr, wgt_late=first
                        )
                        cur_slot = slot
                    fuse = (
                        rep == 0 and i == 0 and nblk > 1 and sched[1][0] == 0
                    )
                    if fuse:
                        # prefetch block 2 mid-phase-A: issuing it at mf 8
                        # keeps its DMA from competing with the startup
                        # w1 stream (the critical path)
                        def hook(mf):
                            if mf == 8 and nblk > 2:
                                load_xg(2)

                        hT0, hT1 = phase_a_multi(
                            [0, 1], w1_ts, [h0_pool, h_pool], mf_hook=hook
                        )
                        phase_b(0, w2_ts, hT0)
                        phase_b(1, w2_ts, hT1)
                        i = 2
                        continue
                    # keep two xg loads in flight (pool bufs=3: blocks
                    # i, i+1, i+2 are live at once)
                    for nxt in (i + 1, i + 2):
                        if nxt < nblk:
                            if nxt not in xg_tiles:
                                load_xg(nxt)
                        elif rep + 1 < reps and nxt == nblk:
                            load_xg(0)
                    hT = phase_a(i, w1_ts, h_pool)
                    phase_b(i, w2_ts, hT)
                    i += 1

    if not nc.is_finalized():
        nc.finalize()
    return nc


def _q8_hilo(v, scale):
    """Quantize v*scale to fp8 e4m3 hi + lo (shared scale), interleaved
    byte-wise on a trailing dim: returns [..., 2] float8_e4m3."""
    import ml_dtypes

    f8 = ml_dtypes.float8_e4m3
    vs = np.asarray(v, np.float32) * np.float32(scale)
    assert np.abs(vs).max() < 240.0, np.abs(vs).max()
    hi = vs.astype(f8)
    lo = (vs - hi.astype(np.float32)).astype(f8)
    return np.ascontiguousarray(np.stack([hi, lo], axis=-1))


def build_program(x, gate_w, w1, w2, top_k):
    x = np.asarray(x, dtype=np.float32)
    gate_w = np.asarray(gate_w, dtype=np.float32)
    w1 = np.asarray(w1, dtype=np.float32)
    w2 = np.asarray(w2, dtype=np.float32)
    assert int(top_k) == 2

    n = x.shape[0]
    ar = np.arange(n)

    # --- host routing (matches reference: softmax -> top2 -> renorm) ---
    logits = (x @ gate_w).astype(np.float64)
    i1 = np.argmax(logits, axis=1)
    lm = logits.copy()
    lm[ar, i1] = -np.inf
    i2 = np.argmax(lm, axis=1)
    m1 = logits[ar, i1]
    m2 = logits[ar, i2]
    g1 = 1.0 / (1.0 + np.exp(m2 - m1))  # = p1/(p1+p2)
    g2 = 1.0 - g1

    gw_full = np.zeros((n, E), dtype=np.float64)
    gw_full[ar, i1] = g1
    gw_full[ar, i2] = g2

    sel = np.zeros((n, E), dtype=bool)
    sel[ar, i1] = True
    sel[ar, i2] = True

    idxs = [np.nonzero(sel[:, e])[0] for e in range(E)]
    counts = np.array([len(ix) for ix in idxs])

    sizes, assign = _optimize_slots(counts)
    K = len(sizes)
    S = sum(sizes)
    bases = np.concatenate([[0], np.cumsum(sizes)]).astype(int)

    # --- bin placement: per slot class, 8 bins assigned to cores in order.
    class_bins = []  # class_bins[k][core] = expert or -1
    for k in range(K):
        lst = []
        for e in range(E):
            lst += [e] * assign[e][k]
        assert len(lst) <= E, (k, lst)
        lst += [-1] * (E - len(lst))
        class_bins.append(lst)

    # expert -> ordered list of (core, slot_base, capacity)
    exp_bins = {e: [] for e in range(E)}
    for k in range(K):
        for c, e in enumerate(class_bins[k]):
            if e >= 0:
                exp_bins[e].append((c, bases[k], sizes[k]))

    # token placement per expert: core_of[e][i], col_of[e][i]
    core_of = {}
    col_of = {}
    per_core_tokens = [[] for _ in range(E)]  # (col_base, tokens, wgts)
    for e in range(E):
        c_e = counts[e]
        core_arr = np.empty(c_e, dtype=np.int64)
        col_arr = np.empty(c_e, dtype=np.int64)
        pos = 0
        for core, base, cap in exp_bins[e]:
            take = min(c_e - pos, cap)
            if take <= 0:
                break
            core_arr[pos : pos + take] = core
            col_arr[pos : pos + take] = base + np.arange(take)
            toks = idxs[e][pos : pos + take]
            per_core_tokens[core].append(
                (base, toks, gw_full[toks, e].astype(np.float32))
            )
            pos += take
        assert pos == c_e, f"expert {e} not fully packed ({pos}/{c_e})"
        core_of[e] = core_arr
        col_of[e] = col_arr

    nc = build_nc(sizes, reps=REPS)

    # per-expert fp8 hi/lo weights, quantized once and shared across bins
    wq_cache = {}

    def wq(e):
        if e not in wq_cache:
            w1q = _q8_hilo(w1[e], SW).reshape(D, 2 * DFF)
            w2q = _q8_hilo(w2[e], SW)  # [DFF, D, 2]
            dh = D // 2
            wq_cache[e] = (
                w1q,
                np.ascontiguousarray(w2q[:, :dh]).reshape(DFF, 2 * dh),
                np.ascontiguousarray(w2q[:, dh:]).reshape(DFF, 2 * dh),
            )
        return wq_cache[e]

    in_maps = []
    for c in range(E):
        xg = np.zeros((S, D), dtype=np.float32)
        wg = np.zeros((S,), dtype=np.float32)
        for base, toks, wvals in per_core_tokens[c]:
            xg[base : base + len(toks)] = x[toks]
            wg[base : base + len(toks)] = wvals
        xqc = _q8_hilo(np.ascontiguousarray(xg.T), SX).reshape(D, 2 * S)
        # fold matmul-2's psum scale into the gate weights
        wgtb = np.broadcast_to(
            (wg * np.float32(2.0**-LOG2_SW))[None, :], (P, S)
        ).copy()
        m = {"xq": xqc, "wgtb": wgtb}
        for k in range(K):
            e_k = class_bins[k][c]
            e_k = e_k if e_k >= 0 else 0
            m[f"w1_{k}"], m[f"w2a_{k}"], m[f"w2b_{k}"] = wq(e_k)
        in_maps.append(m)

    meta = (i1, i2, core_of, col_of)
    return nc, in_maps, meta


def unshard(results, meta):
    i1, i2, core_of, col_of = meta
    n = len(i1)
    ysT = np.stack(
        [np.asarray(results[c]["y"], dtype=np.float32) for c in range(E)]
    )  # [8, D, S]
    c1 = np.empty(n, dtype=np.int64)
    l1 = np.empty(n, dtype=np.int64)
    c2 = np.empty(n, dtype=np.int64)
    l2 = np.empty(n, dtype=np.int64)
    # core_of[e]/col_of[e] are aligned with expert e's ascending token list;
    # recover each token's position in that list via searchsorted.
    ar = np.arange(n)
    selm = np.zeros((n, E), dtype=bool)
    selm[ar, i1] = True
    selm[ar, i2] = True
    for e in range(E):
        toks = np.nonzero(selm[:, e])[0]
        p1 = np.searchsorted(toks, ar[i1 == e])
        c1[i1 == e] = core_of[e][p1]
        l1[i1 == e] = col_of[e][p1]
        p2 = np.searchsorted(toks, ar[i2 == e])
        c2[i2 == e] = core_of[e][p2]
        l2[i2 == e] = col_of[e][p2]
    y = ysT[c1, :, l1] + ysT[c2, :, l2]
    return y.astype(np.float32)


def kernel(x, gate_w, w1, w2, top_k):
    global LAST_RESULT
    nc, in_maps, meta = build_program(x, gate_w, w1, w2, top_k)
    try:
        res = run_bass_kernel_spmd(nc, in_maps, list(range(E)), trace=TRACE)
    except Exception:
        if not TRACE:
            raise
        # tracing unavailable in this environment; rerun untraced
        res = run_bass_kernel_spmd(nc, in_maps, list(range(E)), trace=False)
    global LAST_NC
    LAST_RESULT = res
    LAST_NC = nc
    return unshard(res.results, meta)


# revision 56
# speedup vs baseline: 1.0714x; 1.0714x over previous
"""MoE kernel for Trainium2 (8 NeuronCores, expert-parallel, fp8 DoubleRow).

Strategy
--------
N=8192 tokens, D=1024, E=8 experts, DFF=4096, top_k=2. The reference
computes every expert densely and masks; only each token's top-2 experts
contribute, so we dispatch each token to its 2 experts and run the
expert MLPs on just the routed tokens: 4x fewer FLOPs than dense.

Load balance: expert loads are uneven (1932..2182 here), so instead of
one expert per core (which pads every core to the straggler's 2304
tokens), each core runs K weight slots with compile-time sizes
sum(sizes)=S. The host solves a small covering problem (DP) for the
minimal S such that all experts' token lists pack into 8 bins per slot
class (each bin single-expert); K=3 lands at S=2064 vs the perfect
2048 vs the naive 2304 (-10% PE time).

fp8 DoubleRow matmuls: the PE contracts 2 k-chunks (256 rows) per
DoubleRow instruction at 0.5 cycles per moving row, so a hi+lo fp8
decomposition a@b ~= ah@bh + ah@bl + al@bh (3 e4m3 product terms, the
ll term is ~7e-4 and dropped) runs the same math in 0.75x the bf16
cycles with ~2.7e-3 end-to-end error (bf16 baseline: 3.4e-3). All
hi/lo pairs share one power-of-2 scale per tensor so every term can
accumulate into the same PSUM group: x*32, w1*2048, h*1, w2*2048.
The 2^-16 PSUM scale of matmul 1 is folded into the Silu activation's
input scale; the 2^-11 of matmul 2 is folded into the host-computed
gate weights. h is split on-chip: Silu->bf16 (scalar), Copy->fp8 hh
(scalar), hl = h - hh (DVE, fp8 out).

Both matmuls keep tokens on the PE free dim (phase A: h^T[f,t], phase
B: y^T[d,t]), so block sizes are exact token counts - no 128-row
padding anywhere. The gate weight is applied with a DVE elementwise
multiply against a partition-broadcast copy of the combine weights.

Weights are loaded into SBUF once per slot (graded chunk sizes in
consumption order: small first so the first matmuls start ~5us in,
large after for DMA efficiency; each next slot's load overlaps the
previous slot's trailing phase B). Phase A of the first two blocks is
fused per-mf so the w1 stream keeps up.

Host (unshard): y[token] = yT[core1][:, col1] + yT[core2][:, col2].
"""

import numpy as np

import concourse.bass as bass
import concourse.bacc as bacc
import concourse.tile as tile
from concourse import mybir
from concourse.bass_utils import run_bass_kernel_spmd

N, D, E, DFF = 8192, 1024, 8, 4096
P = 128
KD = D // P  # 8 k-chunks, first matmul
KF = DFF // P  # 32 k-chunks, second matmul
MD = D // P  # 8 output-row tiles, second matmul

# fp8 hi/lo scales (powers of 2; host asserts amax stays under 240)
SX = 32.0  # x scale
SW = 2048.0  # w1 and w2 scale
LOG2_SXW = 16  # log2(SX*SW): psum scale of matmul 1
LOG2_SW = 11  # log2(SW): psum scale of matmul 2

# Per-phase count of chunk-pairs computed with a single pure-fp8 term
# instead of the 3-term hi/lo scheme (speed/accuracy knob; each pure
# pair saves 2 DoubleRow instructions but adds ~5.4e-2*sqrt(frac) err).
PURE_A = 0  # of the 4 k-chunk pairs in matmul 1
PURE_B = 2  # of the 16 k-chunk pairs in matmul 2

TRACE = False
LAST_RESULT = None
LAST_NC = None
REPS = 1  # >1: repeat whole computation in-program (for slope timing)

# schedule tunables (swept offline with TimelineSim; see sweep.py)
XQ_PAIR_ON_SYNC = False  # first fused pair's x DMA queue
GRADES_A = (2, 6, 8)  # w1 first-half DMA chunk sizes (sum 16)
GRADES_B = (8, 8)  # w1 second-half chunk sizes (sum 16)
W2_CHUNKS = 1  # kf-major DMAs per w2 d-half (when W2_DCHUNKS == 1)
W2_DCHUNKS = 2  # d-major DMAs per w2 d-half (256-col min for 512B runs)
XQ2_HOOK_MF = 8  # phase-A mf index at which block 2's x load is issued
FIRST_PAIR = (128, 256)  # fused first blocks
HPS_BUFS = 5  # phase-A PSUM pool bufs (HPS_BUFS + YPS_BUFS <= 8 banks)
YPS_BUFS = 3  # phase-B PSUM pool bufs
PURE_B_AT_END = True  # place the pure kp pairs at the high-kp end
# blocks whose fused-phase-A token total is below this get the hh cast
# on DVE instead of scalar (scalar otherwise paces small-B phase A)
COPY_ON_DVE_MAX_B = 4096
# slot layouts to prefer (TimelineSim-ranked); first feasible one wins
PREFERRED_SIZES = ((512, 992, 560),)


def _chunks_even(total, maxb=512):
    """Split into near-equal blocks <= maxb, multiples of 16 (except possibly
    the last), avoiding tiny tail blocks that expose handoff latency."""
    nb = -(-total // maxb)
    out, rem = [], total
    for i in range(nb):
        b = min(rem, int(np.ceil(rem / (nb - i) / 16) * 16), maxb)
        out.append(b)
        rem -= b
    assert rem == 0 and sum(out) == total
    return out


def _feasible(counts, sizes, n_bins=E, parents=None):
    """DP: can counts be covered with n_bins bins of each size class?
    State: per-class bins used. If parents given, fill for backtracking."""
    K = len(sizes)
    reach = {tuple([0] * K)}
    for e, c in enumerate(counts):
        nxt = set()
        pe = {} if parents is not None else None
        for st in reach:

            def rec(k, st_k, rem):
                if rem <= 0:
                    key = tuple(st_k)
                    if key not in nxt:
                        nxt.add(key)
                        if pe is not None:
                            pe[key] = (st, tuple(np.subtract(st_k, st)))
                    return
                if k == K:
                    return
                for nk in range(n_bins - st_k[k] + 1):
                    st2 = list(st_k)
                    st2[k] += nk
                    rec(k + 1, st2, rem - nk * sizes[k])
                    if nk * sizes[k] >= rem:
                        break

            rec(0, list(st), c)
        if parents is not None:
            parents.append(pe)
        reach = nxt
        if not reach:
            return None
    return next(iter(reach))


def _optimize_slots(counts, n_bins=E):
    """Find slot sizes (K=2, or 3 if strictly better) minimizing
    S = sum(sizes). Returns (sizes, assign) with assign[e][k] = #bins of
    class k used by expert e."""
    counts = np.asarray(counts, dtype=int)
    lo = int(np.ceil(counts.sum() / n_bins / 16) * 16)
    hi = int(np.ceil(counts.max() / 16) * 16) + 16

    def slack_ok(S):
        # zero-slack S needs an exact cover by multiples of 16 => every
        # count must be divisible by 16 (cheap prune of the full scan)
        slack = n_bins * S - int(counts.sum())
        return slack > 0 or all(c % 16 == 0 for c in counts)

    best = None
    S2 = None
    for S in range(lo, 2 * hi, 16):
        if not slack_ok(S):
            continue
        for S_A in range(256, S // 2 + 1, 16):
            if _feasible(counts, (S_A, S - S_A)) is not None:
                best = (S_A, S - S_A)
                break
        if best:
            S2 = S
            break
    assert best is not None, "no 2-slot split found"

    found3 = None
    for S in range(lo, S2, 16):
        if not slack_ok(S):
            continue
        for S_A in range(256, S // 3 + 1, 16):
            for S_B in range(S_A, (S - S_A) // 2 + 1, 16):
                S_C = S - S_A - S_B
                if _feasible(counts, (S_A, S_B, S_C)) is not None:
                    found3 = (S_A, S_B, S_C)
                    break
            if found3:
                break
        if found3:
            break
    sizes = found3 if found3 is not None else best

    # Order slots to maximize the weight-reload windows: the reload of slot
    # k+1 overlaps slot k's LAST block's phase B, so prefer large last
    # blocks on the slots that precede a reload (and a large first slot for
    # the fused start).
    import itertools

    def min_window(order):
        wins = [_chunks_even(order[k])[-1] for k in range(len(order) - 1)]
        return min(wins) if wins else 1 << 30

    sizes = max(
        itertools.permutations(sizes), key=lambda o: (min_window(o), o[0])
    )

    # prefer a TimelineSim-ranked layout when it covers these counts
    for cand in PREFERRED_SIZES:
        if sum(cand) <= sum(sizes) and _feasible(counts, cand) is not None:
            sizes = cand
            break

    parents = []
    assert _feasible(counts, sizes, n_bins, parents) is not None
    assign = [None] * len(counts)
    cur = next(iter(parents[-1]))
    for e in range(len(counts) - 1, -1, -1):
        prev, used = parents[e][cur]
        assign[e] = list(used)
        cur = prev
    return list(sizes), assign


def build_nc(sizes, reps=1):
    """Per-core program: yT[d, t] = wgt[t] * (silu(x @ w1) @ w2)[t, d]
    over len(sizes) weight slots, fp8 DoubleRow hi/lo matmuls."""
    f8 = mybir.dt.float8e4
    bf16 = mybir.dt.bfloat16
    f32 = mybir.dt.float32
    ACT = mybir.ActivationFunctionType
    DR = mybir.MatmulPerfMode.DoubleRow

    K = len(sizes)
    S = sum(sizes)
    nc = bacc.Bacc()
    # All fp8 hi/lo pairs are interleaved byte-wise in the innermost DRAM
    # dim ([..., 2]: 0=hi, 1=lo) so DMA inner contiguous runs stay >=512B
    # (the cost model charges 2x below 512B); matmul APs then read the
    # hi or lo plane with an innermost stride of 2.
    xq = nc.dram_tensor("xq", [D, 2 * S], f8, kind="ExternalInput")
    w1s = [
        nc.dram_tensor(f"w1_{k}", [D, 2 * DFF], f8, kind="ExternalInput")
        for k in range(K)
    ]
    # w2 split into d-halves (separate tensors so the two halves' SBUF
    # tiles free at different times, staggering the next slot's reload)
    DH = D // 2
    w2s = [
        [
            nc.dram_tensor(f"w2{h}_{k}", [DFF, 2 * DH], f8, kind="ExternalInput")
            for h in ("a", "b")
        ]
        for k in range(K)
    ]
    wgtb = nc.dram_tensor("wgtb", [P, S], f32, kind="ExternalInput")
    y = nc.dram_tensor("y", [D, S], bf16, kind="ExternalOutput")

    xq_r = xq.rearrange("(k p) (s two) -> p k s two", p=P, two=2)
    w1_rs = [
        w.rearrange("(k p) (f two) -> p k f two", p=P, two=2) for w in w1s
    ]
    w2_rs = [
        [w.rearrange("(kf p) (d two) -> p kf d two", p=P, two=2) for w in pair]
        for pair in w2s
    ]
    y_r = y.rearrange("(m p) s -> m p s", p=P)  # [8, 128, S]

    # compile-time block schedule: (slot, tok0, B). Slot 0 starts with a
    # small block (its phase A is fused with block 1's, so the PE can start
    # after one small xg DMA + the first w1 chunk). The very last block is
    # small so the end-of-kernel drain waits on a short mult+DMA.
    sched = []
    off = 0
    for s, S_s in enumerate(sizes):
        t0 = off
        if s == 0 and S_s > 512:
            # the fused first pair totals ~320 tokens: the PE consumption
            # rate of the fused phase A then matches the w1 DMA stream
            # rate (2KB/partition per mf chunk), so the start is
            # DMA-paced with no mid-stream stalls
            bs = list(FIRST_PAIR) + _chunks_even(S_s - sum(FIRST_PAIR))
        else:
            bs = _chunks_even(S_s)
        for b in bs:
            sched.append((s, t0, b))
            t0 += b
        off += S_s

    # per-pair term list: hi@hi always; full pairs add lo-weight and
    # lo-moving terms ((j_weight, j_moving) indices into the hi/lo dims)
    TERMS_FULL = ((0, 0), (1, 0), (0, 1))
    TERMS_PURE = ((0, 0),)

    def terms_a(cp):
        return TERMS_PURE if cp < PURE_A else TERMS_FULL

    def terms_b(kp):
        pure = kp >= KF // 2 - PURE_B if PURE_B_AT_END else kp < PURE_B
        return TERMS_PURE if pure else TERMS_FULL

    with tile.TileContext(nc) as tc:
        with (
            tc.tile_pool(name="singles", bufs=1) as singles,
            tc.tile_pool(name="wres", bufs=1) as w_pool,
            tc.tile_pool(name="xg", bufs=3) as xg_pool,
            tc.tile_pool(name="ht", bufs=1) as h_pool,
            tc.tile_pool(name="ht0", bufs=1) as h0_pool,
            tc.tile_pool(name="hbf", bufs=3) as hbf_pool,
            tc.tile_pool(name="yout", bufs=3) as y_pool,
            tc.tile_pool(name="hps", bufs=HPS_BUFS, space="PSUM") as hpsum,
            tc.tile_pool(name="yps", bufs=YPS_BUFS, space="PSUM") as ypsum,
        ):
            xg_tiles = {}  # i -> (tile, col0)

            def load_xg(i, eng=None):
                _, tok0, B = sched[i]
                t = xg_pool.tile([P, KD, B, 2], f8, tag="xg", name="xg")
                (eng or nc.scalar).dma_start(
                    out=t, in_=xq_r[:, :, tok0 : tok0 + B, :]
                )
                xg_tiles[i] = (t, 0)

            def load_xg_pair(i, j, eng):
                """One DMA for two adjacent blocks (shared tile): fewer
                descriptors and a longer (full-rate) inner run."""
                _, t0i, Bi = sched[i]
                _, t0j, Bj = sched[j]
                assert t0j == t0i + Bi
                t = xg_pool.tile([P, KD, Bi + Bj, 2], f8, tag="xg", name="xg")
                eng.dma_start(out=t, in_=xq_r[:, :, t0i : t0i + Bi + Bj, :])
                xg_tiles[i] = (t, 0)
                xg_tiles[j] = (t, Bi)

            def load_weights(slot, xg_loader=None, wgt_late=False):
                """w1 halves first (phase A streams them; graded sizes:
                small first so the PE starts early, >=512B runs after for
                DMA efficiency), then the w2 d-halves (each needed in
                full only by its first phase-B md group); wgtb early in
                the w2 stream. The a/b halves are separate SBUF tiles so
                the next slot's reload of each half starts as soon as the
                previous slot's readers of that half are done (mf<16 /
                md<4 finish well before the slot ends). xg_loader: called
                right after w1's first chunk to slot in the first xg DMA
                (it gates the first matmuls; w1's chunk 0 gates only the
                Ldweights before them)."""
                HF = KF // 2
                w1_ts = []
                for h in range(2):
                    t = w_pool.tile(
                        [P, KD, HF * P, 2], f8, tag=f"w1{h}", name="w1"
                    )
                    # g=2 is the smallest full-rate chunk (512B runs);
                    # few big chunks keep descriptor-generation overhead
                    # (~0.6us per DMA) off the startup critical path
                    grades = GRADES_A if h == 0 else GRADES_B
                    mf = 0
                    for gi, g in enumerate(grades):
                        nc.sync.dma_start(
                            out=t[:, :, mf * P : (mf + g) * P, :],
                            in_=w1_rs[slot][
                                :,
                                :,
                                (h * HF + mf) * P : (h * HF + mf + g) * P,
                                :,
                            ],
                        )
                        mf += g
                        if h == 0 and gi == 0 and xg_loader is not None:
                            xg_loader()
                    assert mf == HF
                    w1_ts.append(t)
                w2_ts = []
                for h in range(2):
                    t = w_pool.tile(
                        [P, KF, DH, 2], f8, tag=f"w2{h}", name="w2"
                    )
                    if W2_DCHUNKS > 1:
                        # d-major chunks match phase B's md consumption
                        # order, letting the first md groups start before
                        # the whole half has landed
                        dchunk = DH // W2_DCHUNKS
                        for c in range(W2_DCHUNKS):
                            nc.sync.dma_start(
                                out=t[:, :, c * dchunk : (c + 1) * dchunk, :],
                                in_=w2_rs[slot][h][
                                    :, :, c * dchunk : (c + 1) * dchunk, :
                                ],
                            )
                    else:
                        kchunk = KF // W2_CHUNKS
                        for c in range(W2_CHUNKS):
                            nc.sync.dma_start(
                                out=t[:, c * kchunk : (c + 1) * kchunk, :, :],
                                in_=w2_rs[slot][h][
                                    :, c * kchunk : (c + 1) * kchunk, :, :
                                ],
                            )
                    if h == 0 and wgt_late:
                        nc.sync.dma_start(out=wgt_t, in_=wgtb[:, :])
                    w2_ts.append(t)
                return w1_ts, w2_ts

            def phase_a_multi(iis, w1_ts, pools, mf_hook=None):
                """Fused phase A over several blocks: per-mf across all
                blocks, so each w1 chunk is consumed at the combined rate
                (lets the first blocks start before w1 fully lands)."""
                xs = [xg_tiles.pop(i) for i in iis]
                hts = []
                for i, pool in zip(iis, pools):
                    _, _, B = sched[i]
                    hts.append(
                        pool.tile(
                            [P, 2, KF, B],
                            f8,
                            tag="ht0" if pool is h0_pool else "ht",
                            name="ht",
                        )
                    )

                def emit(bi, mf):
                    i = iis[bi]
                    _, _, B = sched[i]
                    xt, c0 = xs[bi]
                    w1_t = w1_ts[mf // (KF // 2)]
                    mfl = mf % (KF // 2)
                    ph = hpsum.tile([P, B], f32, tag="hps", name="hps")
                    n_mm = sum(len(terms_a(cp)) for cp in range(KD // 2))
                    mm = 0
                    for cp in range(KD // 2):
                        for jw, jx in terms_a(cp):
                            nc.tensor.matmul(
                                ph[:, :],
                                lhsT=w1_t[
                                    :,
                                    2 * cp : 2 * cp + 2,
                                    mfl * P : (mfl + 1) * P,
                                    jw,
                                ],
                                rhs=xt[
                                    :, 2 * cp : 2 * cp + 2, c0 : c0 + B, jx
                                ],
                                start=(mm == 0),
                                stop=(mm == n_mm - 1),
                                perf_mode=DR,
                            )
                            mm += 1
                    hbf = hbf_pool.tile([P, B], bf16, tag="hbf", name="hbf")
                    nc.scalar.activation(
                        hbf[:, :], ph[:, :], ACT.Silu, scale=2.0**-LOG2_SXW
                    )
                    total_b = sum(sched[i][2] for i in iis)
                    if total_b <= COPY_ON_DVE_MAX_B:
                        nc.vector.tensor_copy(
                            hts[bi][:, 0, mf, :], hbf[:, :]
                        )
                    else:
                        nc.scalar.activation(
                            hts[bi][:, 0, mf, :], hbf[:, :], ACT.Copy
                        )
                    nc.vector.tensor_sub(
                        hts[bi][:, 1, mf, :], hbf[:, :], hts[bi][:, 0, mf, :]
                    )

                for mf in range(KF):
                    if mf_hook is not None:
                        mf_hook(mf)
                    for bi in range(len(iis)):
                        emit(bi, mf)
                return hts

            def phase_a(i, w1_ts, pool):
                return phase_a_multi([i], w1_ts, [pool])[0]

            def phase_b(i, w2_ts, hT):
                _, tok0, B = sched[i]
                for md in range(MD):
                    w2_t = w2_ts[md // (MD // 2)]
                    mdl = md % (MD // 2)
                    yp = ypsum.tile([P, B], f32, tag="yps", name="yps")
                    n_mm = sum(len(terms_b(kp)) for kp in range(KF // 2))
                    mm = 0
                    for kp in range(KF // 2):
                        for jw, jh in terms_b(kp):
                            nc.tensor.matmul(
                                yp[:, :],
                                lhsT=w2_t[
                                    :,
                                    2 * kp : 2 * kp + 2,
                                    mdl * P : (mdl + 1) * P,
                                    jw,
                                ],
                                rhs=hT[:, jh, 2 * kp : 2 * kp + 2, :],
                                start=(mm == 0),
                                stop=(mm == n_mm - 1),
                                perf_mode=DR,
                            )
                            mm += 1
                    y_sb = y_pool.tile([P, B], bf16, tag="yout", name="yout")
                    nc.vector.tensor_mul(
                        y_sb[:, :], yp[:, :], wgt_t[:, tok0 : tok0 + B]
                    )
                    nc.scalar.dma_start(
                        out=y_r[md, :, tok0 : tok0 + B], in_=y_sb[:, :]
                    )

            nblk = len(sched)
            wgt_t = singles.tile([P, S], f32)

            cur_slot = -1
            for rep in range(reps):
                i = 0
                while i < nblk:
                    slot, tok0, B = sched[i]
                    first = cur_slot == -1
                    if slot != cur_slot:
                        if first:
                            def ldr():
                                eng = nc.sync if XQ_PAIR_ON_SYNC else nc.scalar
                                with tc.high_priority():
                                    if nblk > 1 and sched[1][0] == 0:
                                        load_xg_pair(0, 1, eng)
                                    else:
                                        load_xg(0, eng=eng)
                        else:
                            ldr = None
                        w1_ts, w2_ts = load_weights(
                            slot, xg_loader=ldr, wgt_late=first
                        )
                        cur_slot = slot
                    fuse = (
                        rep == 0 and i == 0 and nblk > 1 and sched[1][0] == 0
                    )
                    if fuse:
                        # prefetch block 2 mid-phase-A: issuing it at mf 8
                        # keeps its DMA from competing with the startup
                        # w1 stream (the critical path)
                        def hook(mf):
                            if mf == XQ2_HOOK_MF and nblk > 2:
                                load_xg(2)

                        hT0, hT1 = phase_a_multi(
                            [0, 1], w1_ts, [h0_pool, h_pool], mf_hook=hook
                        )
                        phase_b(0, w2_ts, hT0)
                        phase_b(1, w2_ts, hT1)
                        i = 2
                        continue
                    # keep two xg loads in flight (pool bufs=3: blocks
                    # i, i+1, i+2 are live at once)
                    for nxt in (i + 1, i + 2):
                        if nxt < nblk:
                            if nxt not in xg_tiles:
                                load_xg(nxt)
                        elif rep + 1 < reps and nxt == nblk:
                            load_xg(0)
                    hT = phase_a(i, w1_ts, h_pool)
                    phase_b(i, w2_ts, hT)
                    i += 1

    if not nc.is_finalized():
        nc.finalize()
    return nc


def _q8_hilo(v, scale):
    """Quantize v*scale to fp8 e4m3 hi + lo (shared scale), interleaved
    byte-wise on a trailing dim: returns [..., 2] float8_e4m3."""
    import ml_dtypes

    f8 = ml_dtypes.float8_e4m3
    vs = np.asarray(v, np.float32) * np.float32(scale)
    assert np.abs(vs).max() < 240.0, np.abs(vs).max()
    hi = vs.astype(f8)
    lo = (vs - hi.astype(np.float32)).astype(f8)
    return np.ascontiguousarray(np.stack([hi, lo], axis=-1))


def build_program(x, gate_w, w1, w2, top_k):
    x = np.asarray(x, dtype=np.float32)
    gate_w = np.asarray(gate_w, dtype=np.float32)
    w1 = np.asarray(w1, dtype=np.float32)
    w2 = np.asarray(w2, dtype=np.float32)
    assert int(top_k) == 2

    n = x.shape[0]
    ar = np.arange(n)

    # --- host routing (matches reference: softmax -> top2 -> renorm) ---
    logits = (x @ gate_w).astype(np.float64)
    i1 = np.argmax(logits, axis=1)
    lm = logits.copy()
    lm[ar, i1] = -np.inf
    i2 = np.argmax(lm, axis=1)
    m1 = logits[ar, i1]
    m2 = logits[ar, i2]
    g1 = 1.0 / (1.0 + np.exp(m2 - m1))  # = p1/(p1+p2)
    g2 = 1.0 - g1

    gw_full = np.zeros((n, E), dtype=np.float64)
    gw_full[ar, i1] = g1
    gw_full[ar, i2] = g2

    sel = np.zeros((n, E), dtype=bool)
    sel[ar, i1] = True
    sel[ar, i2] = True

    idxs = [np.nonzero(sel[:, e])[0] for e in range(E)]
    counts = np.array([len(ix) for ix in idxs])

    sizes, assign = _optimize_slots(counts)
    K = len(sizes)
    S = sum(sizes)
    bases = np.concatenate([[0], np.cumsum(sizes)]).astype(int)

    # --- bin placement: per slot class, 8 bins assigned to cores in order.
    class_bins = []  # class_bins[k][core] = expert or -1
    for k in range(K):
        lst = []
        for e in range(E):
            lst += [e] * assign[e][k]
        assert len(lst) <= E, (k, lst)
        lst += [-1] * (E - len(lst))
        class_bins.append(lst)

    # expert -> ordered list of (core, slot_base, capacity)
    exp_bins = {e: [] for e in range(E)}
    for k in range(K):
        for c, e in enumerate(class_bins[k]):
            if e >= 0:
                exp_bins[e].append((c, bases[k], sizes[k]))

    # token placement per expert: core_of[e][i], col_of[e][i]
    core_of = {}
    col_of = {}
    per_core_tokens = [[] for _ in range(E)]  # (col_base, tokens, wgts)
    for e in range(E):
        c_e = counts[e]
        core_arr = np.empty(c_e, dtype=np.int64)
        col_arr = np.empty(c_e, dtype=np.int64)
        pos = 0
        for core, base, cap in exp_bins[e]:
            take = min(c_e - pos, cap)
            if take <= 0:
                break
            core_arr[pos : pos + take] = core
            col_arr[pos : pos + take] = base + np.arange(take)
            toks = idxs[e][pos : pos + take]
            per_core_tokens[core].append(
                (base, toks, gw_full[toks, e].astype(np.float32))
            )
            pos += take
        assert pos == c_e, f"expert {e} not fully packed ({pos}/{c_e})"
        core_of[e] = core_arr
        col_of[e] = col_arr

    nc = build_nc(sizes, reps=REPS)

    # per-expert fp8 hi/lo weights, quantized once and shared across bins
    wq_cache = {}

    def wq(e):
        if e not in wq_cache:
            w1q = _q8_hilo(w1[e], SW).reshape(D, 2 * DFF)
            w2q = _q8_hilo(w2[e], SW)  # [DFF, D, 2]
            dh = D // 2
            wq_cache[e] = (
                w1q,
                np.ascontiguousarray(w2q[:, :dh]).reshape(DFF, 2 * dh),
                np.ascontiguousarray(w2q[:, dh:]).reshape(DFF, 2 * dh),
            )
        return wq_cache[e]

    in_maps = []
    for c in range(E):
        xg = np.zeros((S, D), dtype=np.float32)
        wg = np.zeros((S,), dtype=np.float32)
        for base, toks, wvals in per_core_tokens[c]:
            xg[base : base + len(toks)] = x[toks]
            wg[base : base + len(toks)] = wvals
        xqc = _q8_hilo(np.ascontiguousarray(xg.T), SX).reshape(D, 2 * S)
        # fold matmul-2's psum scale into the gate weights
        wgtb = np.broadcast_to(
            (wg * np.float32(2.0**-LOG2_SW))[None, :], (P, S)
        ).copy()
        m = {"xq": xqc, "wgtb": wgtb}
        for k in range(K):
            e_k = class_bins[k][c]
            e_k = e_k if e_k >= 0 else 0
            m[f"w1_{k}"], m[f"w2a_{k}"], m[f"w2b_{k}"] = wq(e_k)
        in_maps.append(m)

    meta = (i1, i2, core_of, col_of)
    return nc, in_maps, meta


def unshard(results, meta):
    i1, i2, core_of, col_of = meta
    n = len(i1)
    ysT = np.stack(
        [np.asarray(results[c]["y"], dtype=np.float32) for c in range(E)]
    )  # [8, D, S]
    c1 = np.empty(n, dtype=np.int64)
    l1 = np.empty(n, dtype=np.int64)
    c2 = np.empty(n, dtype=np.int64)
    l2 = np.empty(n, dtype=np.int64)
    # core_of[e]/col_of[e] are aligned with expert e's ascending token list;
    # recover each token's position in that list via searchsorted.
    ar = np.arange(n)
    selm = np.zeros((n, E), dtype=bool)
    selm[ar, i1] = True
    selm[ar, i2] = True
    for e in range(E):
        toks = np.nonzero(selm[:, e])[0]
        p1 = np.searchsorted(toks, ar[i1 == e])
        c1[i1 == e] = core_of[e][p1]
        l1[i1 == e] = col_of[e][p1]
        p2 = np.searchsorted(toks, ar[i2 == e])
        c2[i2 == e] = core_of[e][p2]
        l2[i2 == e] = col_of[e][p2]
    y = ysT[c1, :, l1] + ysT[c2, :, l2]
    return y.astype(np.float32)


def kernel(x, gate_w, w1, w2, top_k):
    global LAST_RESULT
    nc, in_maps, meta = build_program(x, gate_w, w1, w2, top_k)
    try:
        res = run_bass_kernel_spmd(nc, in_maps, list(range(E)), trace=TRACE)
    except Exception:
        if not TRACE:
            raise
        # tracing unavailable in this environment; rerun untraced
        res = run_bass_kernel_spmd(nc, in_maps, list(range(E)), trace=False)
    global LAST_NC
    LAST_RESULT = res
    LAST_NC = nc
    return unshard(res.results, meta)


# revision 64
# speedup vs baseline: 1.0985x; 1.0253x over previous
"""MoE kernel for Trainium2 (8 NeuronCores, expert-parallel, fp8 DoubleRow).

Strategy
--------
N=8192 tokens, D=1024, E=8 experts, DFF=4096, top_k=2. The reference
computes every expert densely and masks; only each token's top-2 experts
contribute, so we dispatch each token to its 2 experts and run the
expert MLPs on just the routed tokens: 4x fewer FLOPs than dense.

Load balance: expert loads are uneven (1932..2182 here), so instead of
one expert per core (which pads every core to the straggler's 2304
tokens), each core runs K weight slots with compile-time sizes
sum(sizes)=S. The host solves a small covering problem (DP) for the
minimal S such that all experts' token lists pack into 8 bins per slot
class (each bin single-expert); K=3 lands at S=2064 vs the perfect
2048 vs the naive 2304 (-10% PE time).

fp8 DoubleRow matmuls: the PE contracts 2 k-chunks (256 rows) per
DoubleRow instruction at 0.5 cycles per moving row, so a hi+lo fp8
decomposition a@b ~= ah@bh + ah@bl + al@bh (3 e4m3 product terms, the
ll term is ~7e-4 and dropped) runs the same math in 0.75x the bf16
cycles with ~2.7e-3 end-to-end error (bf16 baseline: 3.4e-3). All
hi/lo pairs share one power-of-2 scale per tensor so every term can
accumulate into the same PSUM group: x*32, w1*2048, h*1, w2*2048.
The 2^-16 PSUM scale of matmul 1 is folded into the Silu activation's
input scale; the 2^-11 of matmul 2 is folded into the host-computed
gate weights. h is split on-chip: Silu->bf16 (scalar), Copy->fp8 hh
(scalar), hl = h - hh (DVE, fp8 out).

Both matmuls keep tokens on the PE free dim (phase A: h^T[f,t], phase
B: y^T[d,t]), so block sizes are exact token counts - no 128-row
padding anywhere. The gate weight is applied with a DVE elementwise
multiply against a partition-broadcast copy of the combine weights.

Weights are loaded into SBUF once per slot (graded chunk sizes in
consumption order: small first so the first matmuls start ~5us in,
large after for DMA efficiency; each next slot's load overlaps the
previous slot's trailing phase B). Phase A of the first two blocks is
fused per-mf so the w1 stream keeps up.

Host (unshard): y[token] = yT[core1][:, col1] + yT[core2][:, col2].
"""

import numpy as np

import concourse.bass as bass
import concourse.bacc as bacc
import concourse.tile as tile
from concourse import mybir
from concourse.bass_utils import run_bass_kernel_spmd

N, D, E, DFF = 8192, 1024, 8, 4096
P = 128
KD = D // P  # 8 k-chunks, first matmul
KF = DFF // P  # 32 k-chunks, second matmul
MD = D // P  # 8 output-row tiles, second matmul

# fp8 hi/lo scales (powers of 2; host asserts amax stays under 240)
SX = 32.0  # x scale
SW = 2048.0  # w1 and w2 scale
LOG2_SXW = 16  # log2(SX*SW): psum scale of matmul 1
LOG2_SW = 11  # log2(SW): psum scale of matmul 2

# Per-phase count of chunk-pairs computed with a single pure-fp8 term
# instead of the 3-term hi/lo scheme (speed/accuracy knob; each pure
# pair saves 2 DoubleRow instructions but adds ~5.4e-2*sqrt(frac) err).
PURE_A = 0  # of the 4 k-chunk pairs in matmul 1
PURE_B = 3  # of the 16 k-chunk pairs in matmul 2

TRACE = False
LAST_RESULT = None
LAST_NC = None
REPS = 1  # >1: repeat whole computation in-program (for slope timing)

# schedule tunables (swept offline with TimelineSim; see sweep.py)
XQ_PAIR_ON_SYNC = False  # first fused pair's x DMA queue
GRADES_A = (2, 2, 2, 2, 4, 4)  # w1 first-half DMA chunk sizes (sum 16)
GRADES_B = (4, 4, 4, 4)  # w1 second-half chunk sizes (sum 16)
W2_CHUNKS = 1  # kf-major DMAs per w2 d-half (when W2_DCHUNKS == 1)
W2_DCHUNKS = 2  # d-major DMAs per w2 d-half (256-col min for 512B runs)
XQ2_HOOK_MF = 8  # phase-A mf index at which block 2's x load is issued
FIRST_PAIR = (128, 256)  # fused first blocks
HPS_BUFS = 5  # phase-A PSUM pool bufs (HPS_BUFS + YPS_BUFS <= 8 banks)
YPS_BUFS = 3  # phase-B PSUM pool bufs
PURE_B_AT_END = True  # place the pure kp pairs at the high-kp end
# blocks whose fused-phase-A token total is below this get the hh cast
# on DVE instead of scalar (scalar otherwise paces small-B phase A)
COPY_ON_DVE_MAX_B = 4096
# slot layouts to prefer (TimelineSim-ranked); first feasible one wins
PREFERRED_SIZES = ((512, 992, 560),)
# optional per-slot block-size override: {slot_index: (b0, b1, ...)}
BLOCKS_OVERRIDE = {}
XG_BUFS = 3  # x tile pool depth (prefetch distance = XG_BUFS - 1)
XG_AHEAD = 2  # blocks of x kept in flight


def _chunks_even(total, maxb=512):
    """Split into near-equal blocks <= maxb, multiples of 16 (except possibly
    the last), avoiding tiny tail blocks that expose handoff latency."""
    nb = -(-total // maxb)
    out, rem = [], total
    for i in range(nb):
        b = min(rem, int(np.ceil(rem / (nb - i) / 16) * 16), maxb)
        out.append(b)
        rem -= b
    assert rem == 0 and sum(out) == total
    return out


def _feasible(counts, sizes, n_bins=E, parents=None):
    """DP: can counts be covered with n_bins bins of each size class?
    State: per-class bins used. If parents given, fill for backtracking."""
    K = len(sizes)
    reach = {tuple([0] * K)}
    for e, c in enumerate(counts):
        nxt = set()
        pe = {} if parents is not None else None
        for st in reach:

            def rec(k, st_k, rem):
                if rem <= 0:
                    key = tuple(st_k)
                    if key not in nxt:
                        nxt.add(key)
                        if pe is not None:
                            pe[key] = (st, tuple(np.subtract(st_k, st)))
                    return
                if k == K:
                    return
                for nk in range(n_bins - st_k[k] + 1):
                    st2 = list(st_k)
                    st2[k] += nk
                    rec(k + 1, st2, rem - nk * sizes[k])
                    if nk * sizes[k] >= rem:
                        break

            rec(0, list(st), c)
        if parents is not None:
            parents.append(pe)
        reach = nxt
        if not reach:
            return None
    return next(iter(reach))


def _optimize_slots(counts, n_bins=E):
    """Find slot sizes (K=2, or 3 if strictly better) minimizing
    S = sum(sizes). Returns (sizes, assign) with assign[e][k] = #bins of
    class k used by expert e."""
    counts = np.asarray(counts, dtype=int)
    lo = int(np.ceil(counts.sum() / n_bins / 16) * 16)
    hi = int(np.ceil(counts.max() / 16) * 16) + 16

    def slack_ok(S):
        # zero-slack S needs an exact cover by multiples of 16 => every
        # count must be divisible by 16 (cheap prune of the full scan)
        slack = n_bins * S - int(counts.sum())
        return slack > 0 or all(c % 16 == 0 for c in counts)

    best = None
    S2 = None
    for S in range(lo, 2 * hi, 16):
        if not slack_ok(S):
            continue
        for S_A in range(256, S // 2 + 1, 16):
            if _feasible(counts, (S_A, S - S_A)) is not None:
                best = (S_A, S - S_A)
                break
        if best:
            S2 = S
            break
    assert best is not None, "no 2-slot split found"

    found3 = None
    for S in range(lo, S2, 16):
        if not slack_ok(S):
            continue
        for S_A in range(256, S // 3 + 1, 16):
            for S_B in range(S_A, (S - S_A) // 2 + 1, 16):
                S_C = S - S_A - S_B
                if _feasible(counts, (S_A, S_B, S_C)) is not None:
                    found3 = (S_A, S_B, S_C)
                    break
            if found3:
                break
        if found3:
            break
    sizes = found3 if found3 is not None else best

    # Order slots to maximize the weight-reload windows: the reload of slot
    # k+1 overlaps slot k's LAST block's phase B, so prefer large last
    # blocks on the slots that precede a reload (and a large first slot for
    # the fused start).
    import itertools

    def min_window(order):
        wins = [_chunks_even(order[k])[-1] for k in range(len(order) - 1)]
        return min(wins) if wins else 1 << 30

    sizes = max(
        itertools.permutations(sizes), key=lambda o: (min_window(o), o[0])
    )

    # prefer a TimelineSim-ranked layout when it covers these counts
    for cand in PREFERRED_SIZES:
        if sum(cand) <= sum(sizes) and _feasible(counts, cand) is not None:
            sizes = cand
            break

    parents = []
    assert _feasible(counts, sizes, n_bins, parents) is not None
    assign = [None] * len(counts)
    cur = next(iter(parents[-1]))
    for e in range(len(counts) - 1, -1, -1):
        prev, used = parents[e][cur]
        assign[e] = list(used)
        cur = prev
    return list(sizes), assign


def build_nc(sizes, reps=1):
    """Per-core program: yT[d, t] = wgt[t] * (silu(x @ w1) @ w2)[t, d]
    over len(sizes) weight slots, fp8 DoubleRow hi/lo matmuls."""
    f8 = mybir.dt.float8e4
    bf16 = mybir.dt.bfloat16
    f32 = mybir.dt.float32
    ACT = mybir.ActivationFunctionType
    DR = mybir.MatmulPerfMode.DoubleRow

    K = len(sizes)
    S = sum(sizes)
    nc = bacc.Bacc()
    # All fp8 hi/lo pairs are interleaved byte-wise in the innermost DRAM
    # dim ([..., 2]: 0=hi, 1=lo) so DMA inner contiguous runs stay >=512B
    # (the cost model charges 2x below 512B); matmul APs then read the
    # hi or lo plane with an innermost stride of 2.
    xq = nc.dram_tensor("xq", [D, 2 * S], f8, kind="ExternalInput")
    w1s = [
        nc.dram_tensor(f"w1_{k}", [D, 2 * DFF], f8, kind="ExternalInput")
        for k in range(K)
    ]
    # w2 split into d-halves (separate tensors so the two halves' SBUF
    # tiles free at different times, staggering the next slot's reload)
    DH = D // 2
    w2s = [
        [
            nc.dram_tensor(f"w2{h}_{k}", [DFF, 2 * DH], f8, kind="ExternalInput")
            for h in ("a", "b")
        ]
        for k in range(K)
    ]
    wgtb = nc.dram_tensor("wgtb", [P, S], f32, kind="ExternalInput")
    y = nc.dram_tensor("y", [D, S], bf16, kind="ExternalOutput")

    xq_r = xq.rearrange("(k p) (s two) -> p k s two", p=P, two=2)
    w1_rs = [
        w.rearrange("(k p) (f two) -> p k f two", p=P, two=2) for w in w1s
    ]
    w2_rs = [
        [w.rearrange("(kf p) (d two) -> p kf d two", p=P, two=2) for w in pair]
        for pair in w2s
    ]
    y_r = y.rearrange("(m p) s -> m p s", p=P)  # [8, 128, S]

    # compile-time block schedule: (slot, tok0, B). Slot 0 starts with a
    # small block (its phase A is fused with block 1's, so the PE can start
    # after one small xg DMA + the first w1 chunk). The very last block is
    # small so the end-of-kernel drain waits on a short mult+DMA.
    sched = []
    off = 0
    for s, S_s in enumerate(sizes):
        t0 = off
        if s in BLOCKS_OVERRIDE:
            bs = list(BLOCKS_OVERRIDE[s])
            assert sum(bs) == S_s, (s, bs, S_s)
        elif s == 0 and S_s > 512:
            # the fused first pair totals ~320 tokens: the PE consumption
            # rate of the fused phase A then matches the w1 DMA stream
            # rate (2KB/partition per mf chunk), so the start is
            # DMA-paced with no mid-stream stalls
            bs = list(FIRST_PAIR) + _chunks_even(S_s - sum(FIRST_PAIR))
        else:
            bs = _chunks_even(S_s)
        for b in bs:
            sched.append((s, t0, b))
            t0 += b
        off += S_s

    # per-pair term list: hi@hi always; full pairs add lo-weight and
    # lo-moving terms ((j_weight, j_moving) indices into the hi/lo dims)
    TERMS_FULL = ((0, 0), (1, 0), (0, 1))
    TERMS_PURE = ((0, 0),)

    def terms_a(cp):
        return TERMS_PURE if cp < PURE_A else TERMS_FULL

    def terms_b(kp):
        pure = kp >= KF // 2 - PURE_B if PURE_B_AT_END else kp < PURE_B
        return TERMS_PURE if pure else TERMS_FULL

    with tile.TileContext(nc) as tc:
        with (
            tc.tile_pool(name="singles", bufs=1) as singles,
            tc.tile_pool(name="wres", bufs=1) as w_pool,
            tc.tile_pool(name="xg", bufs=XG_BUFS) as xg_pool,
            tc.tile_pool(name="ht", bufs=1) as h_pool,
            tc.tile_pool(name="ht0", bufs=1) as h0_pool,
            tc.tile_pool(name="hbf", bufs=3) as hbf_pool,
            tc.tile_pool(name="yout", bufs=3) as y_pool,
            tc.tile_pool(name="hps", bufs=HPS_BUFS, space="PSUM") as hpsum,
            tc.tile_pool(name="yps", bufs=YPS_BUFS, space="PSUM") as ypsum,
        ):
            xg_tiles = {}  # i -> (tile, col0)

            def load_xg(i, eng=None):
                _, tok0, B = sched[i]
                t = xg_pool.tile([P, KD, B, 2], f8, tag="xg", name="xg")
                (eng or nc.scalar).dma_start(
                    out=t, in_=xq_r[:, :, tok0 : tok0 + B, :]
                )
                xg_tiles[i] = (t, 0)

            def load_xg_pair(i, j, eng):
                """One DMA for two adjacent blocks (shared tile): fewer
                descriptors and a longer (full-rate) inner run."""
                _, t0i, Bi = sched[i]
                _, t0j, Bj = sched[j]
                assert t0j == t0i + Bi
                t = xg_pool.tile([P, KD, Bi + Bj, 2], f8, tag="xg", name="xg")
                eng.dma_start(out=t, in_=xq_r[:, :, t0i : t0i + Bi + Bj, :])
                xg_tiles[i] = (t, 0)
                xg_tiles[j] = (t, Bi)

            def load_weights(slot, xg_loader=None, wgt_late=False):
                """w1 halves first (phase A streams them; graded sizes:
                small first so the PE starts early, >=512B runs after for
                DMA efficiency), then the w2 d-halves (each needed in
                full only by its first phase-B md group); wgtb early in
                the w2 stream. The a/b halves are separate SBUF tiles so
                the next slot's reload of each half starts as soon as the
                previous slot's readers of that half are done (mf<16 /
                md<4 finish well before the slot ends). xg_loader: called
                right after w1's first chunk to slot in the first xg DMA
                (it gates the first matmuls; w1's chunk 0 gates only the
                Ldweights before them)."""
                HF = KF // 2
                w1_ts = []
                for h in range(2):
                    t = w_pool.tile(
                        [P, KD, HF * P, 2], f8, tag=f"w1{h}", name="w1"
                    )
                    # g=2 is the smallest full-rate chunk (512B runs);
                    # few big chunks keep descriptor-generation overhead
                    # (~0.6us per DMA) off the startup critical path
                    grades = GRADES_A if h == 0 else GRADES_B
                    mf = 0
                    for gi, g in enumerate(grades):
                        nc.sync.dma_start(
                            out=t[:, :, mf * P : (mf + g) * P, :],
                            in_=w1_rs[slot][
                                :,
                                :,
                                (h * HF + mf) * P : (h * HF + mf + g) * P,
                                :,
                            ],
                        )
                        mf += g
                        if h == 0 and gi == 0 and xg_loader is not None:
                            xg_loader()
                    assert mf == HF
                    w1_ts.append(t)
                w2_ts = []
                for h in range(2):
                    t = w_pool.tile(
                        [P, KF, DH, 2], f8, tag=f"w2{h}", name="w2"
                    )
                    if W2_DCHUNKS > 1:
                        # d-major chunks match phase B's md consumption
                        # order, letting the first md groups start before
                        # the whole half has landed
                        dchunk = DH // W2_DCHUNKS
                        for c in range(W2_DCHUNKS):
                            nc.sync.dma_start(
                                out=t[:, :, c * dchunk : (c + 1) * dchunk, :],
                                in_=w2_rs[slot][h][
                                    :, :, c * dchunk : (c + 1) * dchunk, :
                                ],
                            )
                    else:
                        kchunk = KF // W2_CHUNKS
                        for c in range(W2_CHUNKS):
                            nc.sync.dma_start(
                                out=t[:, c * kchunk : (c + 1) * kchunk, :, :],
                                in_=w2_rs[slot][h][
                                    :, c * kchunk : (c + 1) * kchunk, :, :
                                ],
                            )
                    if h == 0 and wgt_late:
                        nc.sync.dma_start(out=wgt_t, in_=wgtb[:, :])
                    w2_ts.append(t)
                return w1_ts, w2_ts

            def phase_a_multi(iis, w1_ts, pools, mf_hook=None):
                """Fused phase A over several blocks: per-mf across all
                blocks, so each w1 chunk is consumed at the combined rate
                (lets the first blocks start before w1 fully lands)."""
                xs = [xg_tiles.pop(i) for i in iis]
                hts = []
                for i, pool in zip(iis, pools):
                    _, _, B = sched[i]
                    hts.append(
                        pool.tile(
                            [P, 2, KF, B],
                            f8,
                            tag="ht0" if pool is h0_pool else "ht",
                            name="ht",
                        )
                    )

                def emit(bi, mf):
                    i = iis[bi]
                    _, _, B = sched[i]
                    xt, c0 = xs[bi]
                    w1_t = w1_ts[mf // (KF // 2)]
                    mfl = mf % (KF // 2)
                    ph = hpsum.tile([P, B], f32, tag="hps", name="hps")
                    n_mm = sum(len(terms_a(cp)) for cp in range(KD // 2))
                    mm = 0
                    for cp in range(KD // 2):
                        for jw, jx in terms_a(cp):
                            nc.tensor.matmul(
                                ph[:, :],
                                lhsT=w1_t[
                                    :,
                                    2 * cp : 2 * cp + 2,
                                    mfl * P : (mfl + 1) * P,
                                    jw,
                                ],
                                rhs=xt[
                                    :, 2 * cp : 2 * cp + 2, c0 : c0 + B, jx
                                ],
                                start=(mm == 0),
                                stop=(mm == n_mm - 1),
                                perf_mode=DR,
                            )
                            mm += 1
                    hbf = hbf_pool.tile([P, B], bf16, tag="hbf", name="hbf")
                    nc.scalar.activation(
                        hbf[:, :], ph[:, :], ACT.Silu, scale=2.0**-LOG2_SXW
                    )
                    total_b = sum(sched[i][2] for i in iis)
                    if total_b <= COPY_ON_DVE_MAX_B:
                        nc.vector.tensor_copy(
                            hts[bi][:, 0, mf, :], hbf[:, :]
                        )
                    else:
                        nc.scalar.activation(
                            hts[bi][:, 0, mf, :], hbf[:, :], ACT.Copy
                        )
                    nc.vector.tensor_sub(
                        hts[bi][:, 1, mf, :], hbf[:, :], hts[bi][:, 0, mf, :]
                    )

                for mf in range(KF):
                    if mf_hook is not None:
                        mf_hook(mf)
                    for bi in range(len(iis)):
                        emit(bi, mf)
                return hts

            def phase_a(i, w1_ts, pool):
                return phase_a_multi([i], w1_ts, [pool])[0]

            def phase_b(i, w2_ts, hT):
                _, tok0, B = sched[i]
                for md in range(MD):
                    w2_t = w2_ts[md // (MD // 2)]
                    mdl = md % (MD // 2)
                    yp = ypsum.tile([P, B], f32, tag="yps", name="yps")
                    n_mm = sum(len(terms_b(kp)) for kp in range(KF // 2))
                    mm = 0
                    for kp in range(KF // 2):
                        for jw, jh in terms_b(kp):
                            nc.tensor.matmul(
                                yp[:, :],
                                lhsT=w2_t[
                                    :,
                                    2 * kp : 2 * kp + 2,
                                    mdl * P : (mdl + 1) * P,
                                    jw,
                                ],
                                rhs=hT[:, jh, 2 * kp : 2 * kp + 2, :],
                                start=(mm == 0),
                                stop=(mm == n_mm - 1),
                                perf_mode=DR,
                            )
                            mm += 1
                    y_sb = y_pool.tile([P, B], bf16, tag="yout", name="yout")
                    nc.vector.tensor_mul(
                        y_sb[:, :], yp[:, :], wgt_t[:, tok0 : tok0 + B]
                    )
                    nc.scalar.dma_start(
                        out=y_r[md, :, tok0 : tok0 + B], in_=y_sb[:, :]
                    )

            nblk = len(sched)
            wgt_t = singles.tile([P, S], f32)

            cur_slot = -1
            for rep in range(reps):
                i = 0
                while i < nblk:
                    slot, tok0, B = sched[i]
                    first = cur_slot == -1
                    if slot != cur_slot:
                        if first:
                            def ldr():
                                eng = nc.sync if XQ_PAIR_ON_SYNC else nc.scalar
                                with tc.high_priority():
                                    if nblk > 1 and sched[1][0] == 0:
                                        load_xg_pair(0, 1, eng)
                                    else:
                                        load_xg(0, eng=eng)
                        else:
                            ldr = None
                        w1_ts, w2_ts = load_weights(
                            slot, xg_loader=ldr, wgt_late=first
                        )
                        cur_slot = slot
                    fuse = (
                        rep == 0
                        and i == 0
                        and nblk > 1
                        and sched[1][0] == 0
                        and sched[0][2] <= 256  # bounds the ht0 pool size
                    )
                    if fuse:
                        # prefetch block 2 mid-phase-A: issuing it at mf 8
                        # keeps its DMA from competing with the startup
                        # w1 stream (the critical path)
                        def hook(mf):
                            if mf == XQ2_HOOK_MF and nblk > 2:
                                load_xg(2)

                        hT0, hT1 = phase_a_multi(
                            [0, 1], w1_ts, [h0_pool, h_pool], mf_hook=hook
                        )
                        phase_b(0, w2_ts, hT0)
                        phase_b(1, w2_ts, hT1)
                        i = 2
                        continue
                    # keep XG_AHEAD xg loads in flight (pool bufs must
                    # cover blocks i .. i+XG_AHEAD live at once)
                    for nxt in range(i + 1, i + 1 + XG_AHEAD):
                        if nxt < nblk:
                            if nxt not in xg_tiles:
                                load_xg(nxt)
                        elif rep + 1 < reps and nxt == nblk:
                            load_xg(0)
                    hT = phase_a(i, w1_ts, h_pool)
                    phase_b(i, w2_ts, hT)
                    i += 1

    if not nc.is_finalized():
        nc.finalize()
    return nc


def _q8_hilo(v, scale):
    """Quantize v*scale to fp8 e4m3 hi + lo (shared scale), interleaved
    byte-wise on a trailing dim: returns [..., 2] float8_e4m3."""
    import ml_dtypes

    f8 = ml_dtypes.float8_e4m3
    vs = np.asarray(v, np.float32) * np.float32(scale)
    assert np.abs(vs).max() < 240.0, np.abs(vs).max()
    hi = vs.astype(f8)
    lo = (vs - hi.astype(np.float32)).astype(f8)
    return np.ascontiguousarray(np.stack([hi, lo], axis=-1))


def build_program(x, gate_w, w1, w2, top_k):
    x = np.asarray(x, dtype=np.float32)
    gate_w = np.asarray(gate_w, dtype=np.float32)
    w1 = np.asarray(w1, dtype=np.float32)
    w2 = np.asarray(w2, dtype=np.float32)
    assert int(top_k) == 2

    n = x.shape[0]
    ar = np.arange(n)

    # --- host routing (matches reference: softmax -> top2 -> renorm) ---
    logits = (x @ gate_w).astype(np.float64)
    i1 = np.argmax(logits, axis=1)
    lm = logits.copy()
    lm[ar, i1] = -np.inf
    i2 = np.argmax(lm, axis=1)
    m1 = logits[ar, i1]
    m2 = logits[ar, i2]
    g1 = 1.0 / (1.0 + np.exp(m2 - m1))  # = p1/(p1+p2)
    g2 = 1.0 - g1

    gw_full = np.zeros((n, E), dtype=np.float64)
    gw_full[ar, i1] = g1
    gw_full[ar, i2] = g2

    sel = np.zeros((n, E), dtype=bool)
    sel[ar, i1] = True
    sel[ar, i2] = True

    idxs = [np.nonzero(sel[:, e])[0] for e in range(E)]
    counts = np.array([len(ix) for ix in idxs])

    sizes, assign = _optimize_slots(counts)
    K = len(sizes)
    S = sum(sizes)
    bases = np.concatenate([[0], np.cumsum(sizes)]).astype(int)

    # --- bin placement: per slot class, 8 bins assigned to cores in order.
    class_bins = []  # class_bins[k][core] = expert or -1
    for k in range(K):
        lst = []
        for e in range(E):
            lst += [e] * assign[e][k]
        assert len(lst) <= E, (k, lst)
        lst += [-1] * (E - len(lst))
        class_bins.append(lst)

    # expert -> ordered list of (core, slot_base, capacity)
    exp_bins = {e: [] for e in range(E)}
    for k in range(K):
        for c, e in enumerate(class_bins[k]):
            if e >= 0:
                exp_bins[e].append((c, bases[k], sizes[k]))

    # token placement per expert: core_of[e][i], col_of[e][i]
    core_of = {}
    col_of = {}
    per_core_tokens = [[] for _ in range(E)]  # (col_base, tokens, wgts)
    for e in range(E):
        c_e = counts[e]
        core_arr = np.empty(c_e, dtype=np.int64)
        col_arr = np.empty(c_e, dtype=np.int64)
        pos = 0
        for core, base, cap in exp_bins[e]:
            take = min(c_e - pos, cap)
            if take <= 0:
                break
            core_arr[pos : pos + take] = core
            col_arr[pos : pos + take] = base + np.arange(take)
            toks = idxs[e][pos : pos + take]
            per_core_tokens[core].append(
                (base, toks, gw_full[toks, e].astype(np.float32))
            )
            pos += take
        assert pos == c_e, f"expert {e} not fully packed ({pos}/{c_e})"
        core_of[e] = core_arr
        col_of[e] = col_arr

    nc = build_nc(sizes, reps=REPS)

    # per-expert fp8 hi/lo weights, quantized once and shared across bins
    wq_cache = {}

    def wq(e):
        if e not in wq_cache:
            w1q = _q8_hilo(w1[e], SW).reshape(D, 2 * DFF)
            w2q = _q8_hilo(w2[e], SW)  # [DFF, D, 2]
            dh = D // 2
            wq_cache[e] = (
                w1q,
                np.ascontiguousarray(w2q[:, :dh]).reshape(DFF, 2 * dh),
                np.ascontiguousarray(w2q[:, dh:]).reshape(DFF, 2 * dh),
            )
        return wq_cache[e]

    in_maps = []
    for c in range(E):
        xg = np.zeros((S, D), dtype=np.float32)
        wg = np.zeros((S,), dtype=np.float32)
        for base, toks, wvals in per_core_tokens[c]:
            xg[base : base + len(toks)] = x[toks]
            wg[base : base + len(toks)] = wvals
        xqc = _q8_hilo(np.ascontiguousarray(xg.T), SX).reshape(D, 2 * S)
        # fold matmul-2's psum scale into the gate weights
        wgtb = np.broadcast_to(
            (wg * np.float32(2.0**-LOG2_SW))[None, :], (P, S)
        ).copy()
        m = {"xq": xqc, "wgtb": wgtb}
        for k in range(K):
            e_k = class_bins[k][c]
            e_k = e_k if e_k >= 0 else 0
            m[f"w1_{k}"], m[f"w2a_{k}"], m[f"w2b_{k}"] = wq(e_k)
        in_maps.append(m)

    meta = (i1, i2, core_of, col_of)
    return nc, in_maps, meta


def unshard(results, meta):
    i1, i2, core_of, col_of = meta
    n = len(i1)
    ysT = np.stack(
        [np.asarray(results[c]["y"], dtype=np.float32) for c in range(E)]
    )  # [8, D, S]
    c1 = np.empty(n, dtype=np.int64)
    l1 = np.empty(n, dtype=np.int64)
    c2 = np.empty(n, dtype=np.int64)
    l2 = np.empty(n, dtype=np.int64)
    # core_of[e]/col_of[e] are aligned with expert e's ascending token list;
    # recover each token's position in that list via searchsorted.
    ar = np.arange(n)
    selm = np.zeros((n, E), dtype=bool)
    selm[ar, i1] = True
    selm[ar, i2] = True
    for e in range(E):
        toks = np.nonzero(selm[:, e])[0]
        p1 = np.searchsorted(toks, ar[i1 == e])
        c1[i1 == e] = core_of[e][p1]
        l1[i1 == e] = col_of[e][p1]
        p2 = np.searchsorted(toks, ar[i2 == e])
        c2[i2 == e] = core_of[e][p2]
        l2[i2 == e] = col_of[e][p2]
    y = ysT[c1, :, l1] + ysT[c2, :, l2]
    return y.astype(np.float32)


def kernel(x, gate_w, w1, w2, top_k):
    global LAST_RESULT
    nc, in_maps, meta = build_program(x, gate_w, w1, w2, top_k)
    try:
        res = run_bass_kernel_spmd(nc, in_maps, list(range(E)), trace=TRACE)
    except Exception:
        if not TRACE:
            raise
        # tracing unavailable in this environment; rerun untraced
        res = run_bass_kernel_spmd(nc, in_maps, list(range(E)), trace=False)
    global LAST_NC
    LAST_RESULT = res
    LAST_NC = nc
    return unshard(res.results, meta)
